# revision 2
# baseline (speedup 1.0000x reference)
"""Savitzky-Golay filter (window=11, poly=3) on Trainium2 — fp16 I/O version.

Layout strategy (per core: 128 rows x 65536 cols, pure data parallel):
  - Host converts x to fp16; kernel DMAs fp16 both ways -> HBM traffic
    halves vs fp32 (the 2e-2 rel-err budget dwarfs fp16's ~1e-3).
  - DVE 32x32 block-transpose puts time-within-32-blocks on partitions.
  - PE computes the FIR as two fat weight-stationary matmuls per 512-col
    PSUM bank (banded block-diagonal tap matrices W_in / W_next; output
    frame shifted +5 so each output block needs only 2 input blocks).
  - ACT drains PSUM fp32 -> SBUF fp16 WITHOUT un-transposing (plain
    copy-cast); DMA writes the block-transposed result to DRAM.  In the
    drain tail the DVE (done with transposes) takes every other drain.
  - Host undoes the 32x32 block transpose (cheap blocked numpy pass)
    and computes the 10 edge columns exactly in fp32.
  The DVE out-transpose this replaces would cost ~80us (1.04ns/col,
  dtype-independent) and become the bottleneck once DMA drops to ~94us.
"""

from contextlib import ExitStack

import numpy as np

WINDOW = 11
POLY = 3
HALF = WINDOW // 2  # 5
P = 128
L = 65536
N_CORES = 8
ROWS_TOTAL = 1024
ROWS_PER_CORE = ROWS_TOTAL // N_CORES  # 128
B = 32  # DVE stream-transpose block size
BW = 512  # psum bank width (fp32 cols)
PSW = 1024  # psum tile width (2 banks) per ACT drain
CH = 8192  # chunk width (2.1MB fp16 DMAs)
NB = L // B  # 2048 z-blocks per core
N_UNITS = L // CH
# Units from this one on run fine-grained out-DMAs and ACT/DVE-alternating
# drains: by then no in-DMAs remain (no head-of-line risk on the sync
# queue) and the DVE has finished its transposes, so the drain tail runs
# at ~2x a single engine's ~260GB/s and keeps the 16 DMA engines fed.
TAIL = N_UNITS - 1


def _savgol_matrices():
    pos = np.arange(-HALF, HALF + 1, dtype=np.float64)
    A = pos[:, None] ** np.arange(POLY + 1)[None, :]
    c = np.linalg.pinv(A)[0]  # [W] central taps
    V = np.arange(WINDOW, dtype=np.float64)[:, None] ** np.arange(POLY + 1)[None, :]
    T = np.arange(HALF, dtype=np.float64)[:, None] ** np.arange(POLY + 1)[None, :]
    E = T @ np.linalg.pinv(V)  # [HALF, W]
    return c, E


def _build_weights():
    """Stationary lhsT matrices [128, 128] fp32: out[p,f] = sum_q W[q,p] rhs[q,f].

    Block-diagonal over 4 row-blocks (a).  Within a block (i = time-in-block
    of rhs, o = output-time-in-block, +5 frame shift z[j] = y[j+5]):
      W_in[i, o]   = c[i - o]        (0 <= i-o <= 10)
      W_next[i, o] = c[i + 32 - o]   (taps spilling into the next block)
    """
    c64, _ = _savgol_matrices()
    c = c64.astype(np.float32)

    w_in32 = np.zeros((B, B), np.float32)
    w_nx32 = np.zeros((B, B), np.float32)
    for o in range(B):
        for k in range(WINDOW):
            m = o + k
            if m < B:
                w_in32[m, o] = c[k]
            else:
                w_nx32[m - B, o] = c[k]

    def blockdiag(w32):
        W = np.zeros((P, P), np.float32)
        for a in range(P // B):
            W[a * B : (a + 1) * B, a * B : (a + 1) * B] = w32
        return W

    return blockdiag(w_in32), blockdiag(w_nx32)


def _block_t(m):
    """32x32 block transpose of [128, F] (F % 32 == 0)."""
    p, f = m.shape
    v = m.reshape(p // B, B, f // B, B)
    return np.ascontiguousarray(v.transpose(0, 3, 2, 1)).reshape(p, f)


def reference_rows(x):
    c64, E64 = _savgol_matrices()
    c = c64.astype(np.float32)
    E = E64.astype(np.float32)
    R, Lx = x.shape
    out = np.empty_like(x)
    from numpy.lib.stride_tricks import sliding_window_view

    sw = sliding_window_view(x, WINDOW, axis=1)
    out[:, HALF : Lx - HALF] = np.einsum("rlk,k->rl", sw, c, optimize=True).astype(
        np.float32
    )
    out[:, :HALF] = x[:, :WINDOW] @ E.T
    out[:, Lx - HALF :] = (x[:, ::-1][:, :WINDOW] @ E.T)[:, ::-1]
    return out


def simulate_host(x):
    """Numpy simulation of the on-device scheme: returns zdev [P, L] where
    zdev[32a+v, 32m+u] = z[32a+u, 32m+v] and z[r, j] = y[r, j+5]."""
    w_in, w_nx = _build_weights()
    R, Lx = x.shape
    assert R == P and Lx % CH == 0
    zdev = np.zeros_like(x)
    for n in range(Lx // CH):
        base = n * CH
        xin = np.zeros((P, CH + B), np.float32)
        valid = min(CH + B, Lx - base)
        xin[:, :valid] = x[:, base : base + valid]
        btx = _block_t(xin)
        for k in range(CH // BW):
            rhs1 = btx[:, k * BW : k * BW + BW]
            rhs2 = btx[:, k * BW + B : k * BW + BW + B]
            zdev[:, base + k * BW : base + (k + 1) * BW] = (
                w_in.T @ rhs1 + w_nx.T @ rhs2
            )
    return zdev


def _assemble(zdev, xr):
    """Gather step: undo the device's 32-block-transposed z layout and add
    the exact fp32 edge columns.

    zdev: [S, 128, L] with zdev[s, 32a+v, 32m+u] = z[s-slab row 32a+u, 32m+v],
    where z[r, j] = y[r, j+5] (valid for j < L-10).  xr: [S*128, L] fp32.
    """
    S = zdev.shape[0]
    Y = np.empty((S * P, L), np.float32)
    MT = 128  # blocks per inner tile: keeps the strided working set in L2
    nfull = NB - 1  # last block is ragged (only 22 of 32 cols valid)
    for s in range(S):
        for a in range(P // B):
            rows = Y[s * P + a * B : s * P + (a + 1) * B]  # [32, L] view
            zs = zdev[s, a * B : (a + 1) * B]  # [32, L] view
            for t0 in range(0, nfull, MT):
                t1 = min(t0 + MT, nfull)
                dst = rows[:, HALF + t0 * B : HALF + t1 * B].reshape(B, t1 - t0, B)
                src = zs[:, t0 * B : t1 * B].reshape(B, t1 - t0, B)
                dst[:] = src.transpose(2, 1, 0)  # [u,m,v] = [v,m,u]^T
            # ragged tail block: z cols [(NB-1)*32, (NB-1)*32+22) are valid
            v_valid = (L - HALF) - (HALF + nfull * B)  # 22
            blk = zs[:, nfull * B :]  # [32(v), 32(u)]
            rows[:, HALF + nfull * B : L - HALF] = blk[:v_valid, :].T
    _, E64 = _savgol_matrices()
    E = E64.astype(np.float32)
    Y[:, :HALF] = xr[:, :WINDOW] @ E.T
    Y[:, L - HALF :] = (xr[:, ::-1][:, :WINDOW] @ E.T)[:, ::-1]
    return Y


# ---------------------------------------------------------------------------
# Bass kernel
# ---------------------------------------------------------------------------

_NC_CACHE = None


def _build_nc():
    import concourse.tile as tile
    from concourse import bacc, mybir

    assert L % CH == 0 and CH % PSW == 0 and PSW % BW == 0
    nc = bacc.Bacc(
        "TRN2",
        target_bir_lowering=False,
        debug=False,
        enable_asserts=False,
        num_devices=N_CORES,
    )
    f16 = mybir.dt.float16
    f32 = mybir.dt.float32

    x = nc.dram_tensor("x", [P, L], f16, kind="ExternalInput").ap()
    w_in_ap = nc.dram_tensor("w_in", [P, P], f16, kind="ExternalInput").ap()
    w_nx_ap = nc.dram_tensor("w_nx", [P, P], f16, kind="ExternalInput").ap()
    y = nc.dram_tensor("y", [P, L], f16, kind="ExternalOutput").ap()

    n_units = N_UNITS
    npt = CH // PSW

    with tile.TileContext(nc) as tc:
        with ExitStack() as ctx:
            consts = ctx.enter_context(tc.tile_pool(name="consts", bufs=1))
            in_pool = ctx.enter_context(tc.tile_pool(name="inp", bufs=4))
            bt_pool = ctx.enter_context(tc.tile_pool(name="btp", bufs=2))
            out_pool = ctx.enter_context(tc.tile_pool(name="outp", bufs=3))
            ps_pool = ctx.enter_context(tc.tile_pool(name="ps", bufs=3, space="PSUM"))

            def load(n):
                xin = in_pool.tile([P, CH + B], f16, tag="xin")
                base = n * CH
                valid = min(CH + B, L - base)
                nc.sync.dma_start(xin[:, :valid], x[:, base : base + valid])
                if valid < CH + B:
                    nc.gpsimd.memset(xin[:, valid:], 0.0)
                return xin

            def tr(xin):
                btx = bt_pool.tile([P, CH + B], f16, tag="btx")
                nc.vector.transpose(btx[:], xin[:])
                return btx

            # software pipeline: input DMA runs 3 units ahead, the transpose
            # 1 ahead, so no engine queue stalls at its head.
            xins = {0: load(0)}
            wt_in = consts.tile([P, P], f16, tag="w_in")
            nc.sync.dma_start(wt_in[:], w_in_ap)
            wt_nx = consts.tile([P, P], f16, tag="w_nx")
            nc.sync.dma_start(wt_nx[:], w_nx_ap)
            for i in range(1, min(3, n_units)):
                xins[i] = load(i)
            btxs = {0: tr(xins.pop(0))}

            for n in range(n_units):
                base = n * CH
                btx = btxs.pop(n)
                zst = out_pool.tile([P, CH], f16, tag="zst")
                tail = n >= TAIL

                # psum tiles in pairs; all w_in matmuls for the pair, then all
                # w_nx: 2 stationary loads per 2048 output cols instead of 4.
                lo = 0  # next un-DMA'd zst column
                for kp in range(0, npt, 2):
                    ps_a = ps_pool.tile([P, PSW], f32, tag="ps")
                    ps_b = ps_pool.tile([P, PSW], f32, tag="ps")
                    pair = [ps_a, ps_b]
                    for wt, st, sp, off in (
                        (wt_in, True, False, 0),
                        (wt_nx, False, True, B),
                    ):
                        for i, k in enumerate((kp, kp + 1)):
                            for h in range(PSW // BW):
                                c0 = k * PSW + h * BW + off
                                nc.tensor.matmul(
                                    pair[i][:, h * BW : h * BW + BW],
                                    wt[:],
                                    btx[:, c0 : c0 + BW],
                                    start=st,
                                    stop=sp,
                                )
                    if kp == 0:
                        # queue next unit's input work ahead of the drains
                        if n + 3 < n_units:
                            xins[n + 3] = load(n + 3)
                        if n + 1 < n_units:
                            btxs[n + 1] = tr(xins.pop(n + 1))
                    for i, k in enumerate((kp, kp + 1)):
                        if tail and k % 2 == 1:
                            nc.vector.tensor_copy(
                                zst[:, k * PSW : (k + 1) * PSW], pair[i][:]
                            )
                        else:
                            nc.scalar.copy(
                                zst[:, k * PSW : (k + 1) * PSW], pair[i][:]
                            )
                    if tail:
                        # drain tail: fine-grained out-DMAs keep engines fed
                        hi = (kp + 2) * PSW
                        nc.sync.dma_start(
                            y[:, base + lo : base + hi], zst[:, lo:hi]
                        )
                        lo = hi
                    elif kp == npt // 2 - 2:
                        nc.sync.dma_start(
                            y[:, base : base + (npt // 2) * PSW],
                            zst[:, : (npt // 2) * PSW],
                        )
                        lo = (npt // 2) * PSW
                if lo < CH:
                    nc.sync.dma_start(
                        y[:, base + lo : base + CH], zst[:, lo:]
                    )

    nc.compile()
    return nc


def _get_nc():
    global _NC_CACHE
    if _NC_CACHE is None:
        _NC_CACHE = _build_nc()
    return _NC_CACHE


def _in_maps(x: np.ndarray) -> list[dict]:
    w_in, w_nx = (w.astype(np.float16) for w in _build_weights())
    xr = np.ascontiguousarray(x.reshape(ROWS_TOTAL, L)).astype(np.float16)
    return [
        {
            "x": xr[i * ROWS_PER_CORE : (i + 1) * ROWS_PER_CORE],
            "w_in": w_in,
            "w_nx": w_nx,
        }
        for i in range(N_CORES)
    ]


def kernel(x: np.ndarray) -> np.ndarray:
    from concourse.bass_utils import run_bass_kernel_spmd

    assert x.shape == (64, 16, L) and x.dtype == np.float32
    nc = _get_nc()
    in_maps = _in_maps(x)
    res = run_bass_kernel_spmd(nc, in_maps, core_ids=list(range(N_CORES)))
    zdev = np.stack([r["y"] for r in res.results])  # [8, 128, L] fp16
    xr = np.ascontiguousarray(x.reshape(ROWS_TOTAL, L))
    return _assemble(zdev, xr).reshape(64, 16, L)


if __name__ == "__main__":
    rng = np.random.default_rng(0)
    xt = rng.standard_normal((P, L)).astype(np.float32)
    ref = reference_rows(xt)
    zdev = simulate_host(xt)
    out = _assemble(zdev[None], xt)
    err = np.abs(out - ref).max()
    rel = err / np.abs(ref).max()
    print(f"host sim vs ref: max abs {err:.3e}  rel {rel:.3e}")


# revision 5
# speedup vs baseline: 1.1274x; 1.1274x over previous
"""Savitzky-Golay filter (window=11, poly=3) on Trainium2 — fp16 I/O version.

Layout strategy (per core: 128 rows x 65536 cols, pure data parallel):
  - Host converts x to fp16; kernel DMAs fp16 both ways -> HBM traffic
    halves vs fp32 (the 2e-2 rel-err budget dwarfs fp16's ~1e-3).
  - DVE 32x32 block-transpose puts time-within-32-blocks on partitions.
  - PE computes the FIR as two fat weight-stationary matmuls per 512-col
    PSUM bank (banded block-diagonal tap matrices W_in / W_next; output
    frame shifted +5 so each output block needs only 2 input blocks).
  - ACT drains PSUM fp32 -> SBUF fp16 WITHOUT un-transposing (plain
    copy-cast); DMA writes the block-transposed result to DRAM.  In the
    drain tail the DVE (done with transposes) takes every other drain.
  - Host undoes the 32x32 block transpose (cheap blocked numpy pass)
    and computes the 10 edge columns exactly in fp32.
  The DVE out-transpose this replaces would cost ~80us (1.04ns/col,
  dtype-independent) and become the bottleneck once DMA drops to ~94us.
"""

from contextlib import ExitStack

import numpy as np

WINDOW = 11
POLY = 3
HALF = WINDOW // 2  # 5
P = 128
L = 65536
N_CORES = 8
ROWS_TOTAL = 1024
ROWS_PER_CORE = ROWS_TOTAL // N_CORES  # 128
B = 32  # DVE stream-transpose block size
BW = 512  # psum bank width (fp32 cols)
PSW = 1024  # psum tile width (2 banks) per ACT drain
CH = 8192  # chunk width (2.1MB fp16 DMAs)
NB = L // B  # 2048 z-blocks per core
N_UNITS = L // CH
# Units from this one on run fine-grained out-DMAs and ACT/DVE-alternating
# drains: by then no in-DMAs remain (no head-of-line risk on the sync
# queue) and the DVE has finished its transposes, so the drain tail runs
# at ~2x a single engine's ~260GB/s and keeps the 16 DMA engines fed.
TAIL = N_UNITS - 1


def _savgol_matrices():
    pos = np.arange(-HALF, HALF + 1, dtype=np.float64)
    A = pos[:, None] ** np.arange(POLY + 1)[None, :]
    c = np.linalg.pinv(A)[0]  # [W] central taps
    V = np.arange(WINDOW, dtype=np.float64)[:, None] ** np.arange(POLY + 1)[None, :]
    T = np.arange(HALF, dtype=np.float64)[:, None] ** np.arange(POLY + 1)[None, :]
    E = T @ np.linalg.pinv(V)  # [HALF, W]
    return c, E


def _build_weights():
    """Stationary lhsT matrices [128, 128] fp32: out[p,f] = sum_q W[q,p] rhs[q,f].

    Block-diagonal over 4 row-blocks (a).  Within a block (i = time-in-block
    of rhs, o = output-time-in-block, +5 frame shift z[j] = y[j+5]):
      W_in[i, o]   = c[i - o]        (0 <= i-o <= 10)
      W_next[i, o] = c[i + 32 - o]   (taps spilling into the next block)
    """
    c64, _ = _savgol_matrices()
    c = c64.astype(np.float32)

    w_in32 = np.zeros((B, B), np.float32)
    w_nx32 = np.zeros((B, B), np.float32)
    for o in range(B):
        for k in range(WINDOW):
            m = o + k
            if m < B:
                w_in32[m, o] = c[k]
            else:
                w_nx32[m - B, o] = c[k]

    def blockdiag(w32):
        W = np.zeros((P, P), np.float32)
        for a in range(P // B):
            W[a * B : (a + 1) * B, a * B : (a + 1) * B] = w32
        return W

    return blockdiag(w_in32), blockdiag(w_nx32)


def _block_t(m):
    """32x32 block transpose of [128, F] (F % 32 == 0)."""
    p, f = m.shape
    v = m.reshape(p // B, B, f // B, B)
    return np.ascontiguousarray(v.transpose(0, 3, 2, 1)).reshape(p, f)


def reference_rows(x):
    c64, E64 = _savgol_matrices()
    c = c64.astype(np.float32)
    E = E64.astype(np.float32)
    R, Lx = x.shape
    out = np.empty_like(x)
    from numpy.lib.stride_tricks import sliding_window_view

    sw = sliding_window_view(x, WINDOW, axis=1)
    out[:, HALF : Lx - HALF] = np.einsum("rlk,k->rl", sw, c, optimize=True).astype(
        np.float32
    )
    out[:, :HALF] = x[:, :WINDOW] @ E.T
    out[:, Lx - HALF :] = (x[:, ::-1][:, :WINDOW] @ E.T)[:, ::-1]
    return out


def simulate_host(x):
    """Numpy simulation of the on-device scheme: returns zdev [P, L] where
    zdev[32a+v, 32m+u] = z[32a+u, 32m+v] and z[r, j] = y[r, j+5]."""
    w_in, w_nx = _build_weights()
    R, Lx = x.shape
    assert R == P and Lx % CH == 0
    zdev = np.zeros_like(x)
    for n in range(Lx // CH):
        base = n * CH
        xin = np.zeros((P, CH + B), np.float32)
        valid = min(CH + B, Lx - base)
        xin[:, :valid] = x[:, base : base + valid]
        btx = _block_t(xin)
        for k in range(CH // BW):
            rhs1 = btx[:, k * BW : k * BW + BW]
            rhs2 = btx[:, k * BW + B : k * BW + BW + B]
            zdev[:, base + k * BW : base + (k + 1) * BW] = (
                w_in.T @ rhs1 + w_nx.T @ rhs2
            )
    return zdev


def _assemble(zdev, xr):
    """Gather step: undo the device's 32-block-transposed z layout and add
    the exact fp32 edge columns.

    zdev: [S, 128, L] with zdev[s, 32a+v, 32m+u] = z[s-slab row 32a+u, 32m+v],
    where z[r, j] = y[r, j+5] (valid for j < L-10).  xr: [S*128, L] fp32.
    """
    S = zdev.shape[0]
    Y = np.empty((S * P, L), np.float32)
    MT = 128  # blocks per inner tile: keeps the strided working set in L2
    nfull = NB - 1  # last block is ragged (only 22 of 32 cols valid)
    for s in range(S):
        for a in range(P // B):
            rows = Y[s * P + a * B : s * P + (a + 1) * B]  # [32, L] view
            zs = zdev[s, a * B : (a + 1) * B]  # [32, L] view
            for t0 in range(0, nfull, MT):
                t1 = min(t0 + MT, nfull)
                dst = rows[:, HALF + t0 * B : HALF + t1 * B].reshape(B, t1 - t0, B)
                src = zs[:, t0 * B : t1 * B].reshape(B, t1 - t0, B)
                dst[:] = src.transpose(2, 1, 0)  # [u,m,v] = [v,m,u]^T
            # ragged tail block: z cols [(NB-1)*32, (NB-1)*32+22) are valid
            v_valid = (L - HALF) - (HALF + nfull * B)  # 22
            blk = zs[:, nfull * B :]  # [32(v), 32(u)]
            rows[:, HALF + nfull * B : L - HALF] = blk[:v_valid, :].T
    _, E64 = _savgol_matrices()
    E = E64.astype(np.float32)
    Y[:, :HALF] = xr[:, :WINDOW] @ E.T
    Y[:, L - HALF :] = (xr[:, ::-1][:, :WINDOW] @ E.T)[:, ::-1]
    return Y


# ---------------------------------------------------------------------------
# Bass kernel
# ---------------------------------------------------------------------------

_NC_CACHE = None


def _build_nc():
    import concourse.tile as tile
    from concourse import bacc, mybir

    assert L % CH == 0 and CH % PSW == 0 and PSW % BW == 0
    nc = bacc.Bacc(
        "TRN2",
        target_bir_lowering=False,
        debug=False,
        enable_asserts=False,
        num_devices=N_CORES,
    )
    f16 = mybir.dt.float16
    f32 = mybir.dt.float32

    x = nc.dram_tensor("x", [P, L], f16, kind="ExternalInput").ap()
    w_in_ap = nc.dram_tensor("w_in", [P, P], f16, kind="ExternalInput").ap()
    w_nx_ap = nc.dram_tensor("w_nx", [P, P], f16, kind="ExternalInput").ap()
    y = nc.dram_tensor("y", [P, L], f16, kind="ExternalOutput").ap()

    n_units = N_UNITS
    npt = CH // PSW

    with tile.TileContext(nc) as tc:
        with ExitStack() as ctx:
            consts = ctx.enter_context(tc.tile_pool(name="consts", bufs=1))
            in_pool = ctx.enter_context(tc.tile_pool(name="inp", bufs=4))
            bt_pool = ctx.enter_context(tc.tile_pool(name="btp", bufs=2))
            out_pool = ctx.enter_context(tc.tile_pool(name="outp", bufs=3))
            ps_pool = ctx.enter_context(tc.tile_pool(name="ps", bufs=3, space="PSUM"))

            # matmul pair kp reads btx cols up to (kp+2)*PSW + B, so these
            # split points let compute start as soon as a piece is transposed
            SPLIT0 = 2 * PSW + B  # 2080
            QUARTERS = (2 * PSW + B, 4 * PSW + B, 6 * PSW + B, CH + B)

            def load(n):
                xin = in_pool.tile([P, CH + B], f16, tag="xin")
                base = n * CH
                valid = min(CH + B, L - base)
                if n == 0:
                    # split the first load so the DVE transpose chain (the
                    # serial in-side resource) starts ~5us earlier
                    nc.sync.dma_start(xin[:, :SPLIT0], x[:, :SPLIT0])
                    nc.sync.dma_start(
                        xin[:, SPLIT0:valid], x[:, SPLIT0:valid]
                    )
                else:
                    nc.sync.dma_start(xin[:, :valid], x[:, base : base + valid])
                if valid < CH + B:
                    nc.gpsimd.memset(xin[:, valid:], 0.0)
                return xin

            def tr(xin, splits=(CH + B,)):
                btx = bt_pool.tile([P, CH + B], f16, tag="btx")
                lo = 0
                for hi in splits:
                    nc.vector.transpose(btx[:, lo:hi], xin[:, lo:hi])
                    lo = hi
                return btx

            # software pipeline: input DMA runs 3 units ahead, the transpose
            # 1 ahead, so no engine queue stalls at its head.
            xins = {0: load(0)}
            wt_in = consts.tile([P, P], f16, tag="w_in")
            nc.sync.dma_start(wt_in[:], w_in_ap)
            wt_nx = consts.tile([P, P], f16, tag="w_nx")
            nc.sync.dma_start(wt_nx[:], w_nx_ap)
            for i in range(1, min(3, n_units)):
                xins[i] = load(i)
            btxs = {0: tr(xins.pop(0), splits=(SPLIT0, CH + B))}

            for n in range(n_units):
                base = n * CH
                btx = btxs.pop(n)
                zst = out_pool.tile([P, CH], f16, tag="zst")
                tail = n >= TAIL

                # psum tiles in pairs; all w_in matmuls for the pair, then all
                # w_nx: 2 stationary loads per 2048 output cols instead of 4.
                lo = 0  # next un-DMA'd zst column
                for kp in range(0, npt, 2):
                    ps_a = ps_pool.tile([P, PSW], f32, tag="ps")
                    ps_b = ps_pool.tile([P, PSW], f32, tag="ps")
                    pair = [ps_a, ps_b]
                    for wt, st, sp, off in (
                        (wt_in, True, False, 0),
                        (wt_nx, False, True, B),
                    ):
                        for i, k in enumerate((kp, kp + 1)):
                            for h in range(PSW // BW):
                                c0 = k * PSW + h * BW + off
                                nc.tensor.matmul(
                                    pair[i][:, h * BW : h * BW + BW],
                                    wt[:],
                                    btx[:, c0 : c0 + BW],
                                    start=st,
                                    stop=sp,
                                )
                    if kp == 0:
                        # queue next unit's input work ahead of the drains
                        if n + 3 < n_units:
                            xins[n + 3] = load(n + 3)
                        if n + 1 < n_units:
                            # the last unit's transpose lands in quarters so
                            # its matmuls/drains chase the pieces instead of
                            # idling for the full 8.7us transpose
                            btxs[n + 1] = tr(
                                xins.pop(n + 1),
                                splits=QUARTERS
                                if n + 1 == n_units - 1
                                else (CH + B,),
                            )
                    for i, k in enumerate((kp, kp + 1)):
                        if tail and k % 2 == 1:
                            nc.vector.tensor_copy(
                                zst[:, k * PSW : (k + 1) * PSW], pair[i][:]
                            )
                        else:
                            nc.scalar.copy(
                                zst[:, k * PSW : (k + 1) * PSW], pair[i][:]
                            )
                    if tail:
                        # drain tail: fine-grained out-DMAs keep engines fed
                        hi = (kp + 2) * PSW
                        nc.sync.dma_start(
                            y[:, base + lo : base + hi], zst[:, lo:hi]
                        )
                        lo = hi
                    elif kp == npt // 2 - 2:
                        nc.sync.dma_start(
                            y[:, base : base + (npt // 2) * PSW],
                            zst[:, : (npt // 2) * PSW],
                        )
                        lo = (npt // 2) * PSW
                if lo < CH:
                    nc.sync.dma_start(
                        y[:, base + lo : base + CH], zst[:, lo:]
                    )

    nc.compile()
    return nc


def _get_nc():
    global _NC_CACHE
    if _NC_CACHE is None:
        _NC_CACHE = _build_nc()
    return _NC_CACHE


def _in_maps(x: np.ndarray) -> list[dict]:
    w_in, w_nx = (w.astype(np.float16) for w in _build_weights())
    xr = np.ascontiguousarray(x.reshape(ROWS_TOTAL, L)).astype(np.float16)
    return [
        {
            "x": xr[i * ROWS_PER_CORE : (i + 1) * ROWS_PER_CORE],
            "w_in": w_in,
            "w_nx": w_nx,
        }
        for i in range(N_CORES)
    ]


def kernel(x: np.ndarray) -> np.ndarray:
    from concourse.bass_utils import run_bass_kernel_spmd

    assert x.shape == (64, 16, L) and x.dtype == np.float32
    nc = _get_nc()
    in_maps = _in_maps(x)
    res = run_bass_kernel_spmd(nc, in_maps, core_ids=list(range(N_CORES)))
    zdev = np.stack([r["y"] for r in res.results])  # [8, 128, L] fp16
    xr = np.ascontiguousarray(x.reshape(ROWS_TOTAL, L))
    return _assemble(zdev, xr).reshape(64, 16, L)


if __name__ == "__main__":
    rng = np.random.default_rng(0)
    xt = rng.standard_normal((P, L)).astype(np.float32)
    ref = reference_rows(xt)
    zdev = simulate_host(xt)
    out = _assemble(zdev[None], xt)
    err = np.abs(out - ref).max()
    rel = err / np.abs(ref).max()
    print(f"host sim vs ref: max abs {err:.3e}  rel {rel:.3e}")
